# revision 7
# baseline (speedup 1.0000x reference)
"""CostVolumeLayer Trainium2 kernel (v2).

Computes the local cost volume: for search_range R=4,
  out[b, di*9+dj, i, j] = sum_c src[b,c,i,j] * tgt_zp[b,c,i-2R+di, j-2R+dj]
(tgt zero-padded outside its bounds; the window is OFF-CENTER, covering
tgt rows i-8..i and cols j-8..j — faithful to the torch reference).

Strategy (8 NeuronCores, SPMD):
  - Shard: core c -> batch b = c//2, row-half r0 = 32*(c%2). Each core gets
    src shard [C=128, 32, 128] and a zero-padded tgt halo shard
    [C=128, 40, 136] (host pre-pads; halo = R rows/cols each side).
  - Device: for each 8x16 pixel block, TWO accumulating TensorE matmuls
    build the banded Gram directly with full 128-partition occupancy:
      A: stationary [pixels mi<4 (64) | zeros (64)],  rhs = window rows
         [8bi, 8bi+12) x 24 cols  (N=288)  -> start=True
      B: stationary [zeros (64) | pixels mi>=4 (64)], rhs = rows
         [8bi+4, 8bi+16) x 24    (N=288)  -> start=False (accumulate)
    Lower pixels get Gram cols for window rows 0..11, upper pixels rows
    4..15 (shifted by -96 cols) — exactly the baseline band layout, but
    produced in ONE psum bank at full partition width. The host gather is
    unchanged. PE cost/block: 2x288 streamed cols (vs 384 + 64-partition
    copies in v1); copies run at full partition width, halving ACT/DVE
    evacuation time, which was the v1 bottleneck.
  - PSUM: 4-bank tiles (4 blocks each), 2 bufs = all 8 banks. One
    fp32->fp16 copy per tile [128, 4x288] (DVE and ACT split 5/3).
  - DMA: inputs and outputs split across BOTH hardware DGE queues
    (Sync + Scalar) instead of v1's single Sync queue: src chunks +
    2 output stages on Scalar, tgt chunks + 2 output stages on Sync.
  - Stage/out: 2 psum tiles -> one SBUF stage [128, 2304] fp16 -> one
    DMA; 4 stages total (one per block-row).
  - Host: zero-FLOP banded-diagonal gather (identical to v1).
"""

import numpy as np

R = 4
D = 2 * R + 1          # 9
B, C, H, W = 4, 128, 64, 128
NCORES = 8
HS = H // 2            # 32 rows per core shard
TH = HS + 2 * R        # 40 padded tgt rows per shard
TW = W + 2 * R         # 136 padded tgt cols
BI, BJ = 8, 16         # pixel block: 8 rows x 16 cols = 128 = M
NBI, NBJ = HS // BI, W // BJ   # 4 x 8 = 32 blocks per core
WIN_J = BJ + 2 * R     # 24 window cols
NA = 12 * WIN_J        # 288 streamed cols per half-matmul (12 window rows)
BANDW = NA             # 288 band cols dumped per pixel
BANDO = 4 * WIN_J      # 96, upper-half band column offset (host gather)
BLKC = 192             # src cols per block: [pixA 64 | zeros 64 | pixB 64]
SRCC = NBI * NBJ * BLKC  # 6144
TGTC = TH * TW           # 5440
E = SRCC + TGTC          # 11584 input cols per partition
PSB = 512              # fp32 elems per PSUM bank (2KB)
TPB = 4                # blocks (banks) per psum tile
STGB = 8               # blocks per output stage/DMA
STGW = STGB * BANDW    # 2304 fp16 cols per stage

_compiled = None


def _build_bass():
    import concourse.mybir as mybir
    from concourse import bacc
    from concourse.tile import TileContext

    f32 = mybir.dt.float32
    in_dt = mybir.dt.bfloat16
    dump_dt = mybir.dt.float16
    nc = bacc.Bacc()
    inp = nc.dram_tensor("inp", [C, E], in_dt, kind="ExternalInput")
    gout = nc.dram_tensor("gout", [NBI, 128, STGW], dump_dt,
                          kind="ExternalOutput")
    gout_ap = gout.ap()

    with TileContext(nc) as tc:
        with (
            tc.tile_pool(name="inp", bufs=1) as inp_pool,
            tc.tile_pool(name="g", bufs=NBI) as gpool,
            tc.tile_pool(name="psum", bufs=2, space="PSUM") as psum_pool,
        ):
            a = inp_pool.tile([C, E], in_dt)

            def t_view():
                return a[:, SRCC:].rearrange("c (i j) -> c i j", j=TW)

            # PE warm-up: the HAM clock gate needs sustained PE activity to
            # ramp to full clock; run dummy N=512 matmuls during the input
            # DMA wait. The warm psum tile is main-pool buffer 0, recycled
            # by the second real tile once the warm-ups retire.
            warm = inp_pool.tile([128, PSB], in_dt)
            nc.gpsimd.memset(warm, 0.0)

            def new_pt():
                # single call site: the pool reserves bufs x one tile size
                return psum_pool.tile([128, TPB * PSB], f32, name="pt")

            # Warm matmuls write into the first real psum tile (bank 0);
            # block 0's start=True matmul overwrites it afterwards (PE
            # program order keeps this safe).
            wps = new_pt()
            for _ in range(6):
                nc.tensor.matmul(wps[0:1, 0:PSB], warm[:, :1], warm,
                                 start=True, stop=True)
            # ACT warm-up: first Activation op loads the activation table
            # (~1.3us); pay it during the input wait.
            actwarm = inp_pool.tile([1, 1], dump_dt)
            nc.scalar.copy(actwarm, warm[0:1, 0:1])

            # Input DMAs split across both HWDGE queues, issued in
            # consumption order. Scalar queue: src block-rows. Sync queue:
            # tgt row-chunks.
            iv = inp.ap()
            SRCR = NBJ * BLKC  # 1536 src cols per block-row

            def src_chunk(q, lo, hi):
                q.dma_start(out=a[:, lo * SRCR:hi * SRCR],
                            in_=iv[:, lo * SRCR:hi * SRCR])

            def tgt_chunk(q, r0, r1):
                lo, hi = SRCC + r0 * TW, SRCC + r1 * TW
                q.dma_start(out=a[:, lo:hi], in_=iv[:, lo:hi])

            src_chunk(nc.scalar, 0, 1)
            tgt_chunk(nc.sync, 0, 12)
            src_chunk(nc.scalar, 1, 4)
            tgt_chunk(nc.sync, 12, 20)
            tgt_chunk(nc.sync, 20, 28)
            tgt_chunk(nc.sync, 28, 40)

            # Copy engines per psum tile (DVE 5 / ACT 3, interleaved).
            COPY_ENG = [0, 1, 0, 0, 1, 0, 0, 1]  # 0=DVE, 1=ACT

            for bi in range(NBI):
                stage = gpool.tile([128, STGW], dump_dt)
                for h in range(2):
                    pt = wps if bi == 0 and h == 0 else new_pt()
                    ptv = pt.rearrange("p (b h) -> p b h", b=TPB)
                    for j in range(TPB):
                        blk = bi * NBJ + h * TPB + j
                        bj = blk % NBJ
                        sb = blk * BLKC
                        lhsA = a[:, sb:sb + 128]          # [pixA | z]
                        lhsB = a[:, sb + 64:sb + 192]     # [z | pixB]
                        rhsA = t_view()[:, bi * BI: bi * BI + 12,
                                        bj * BJ: bj * BJ + WIN_J]
                        rhsB = t_view()[:, bi * BI + 4: bi * BI + 16,
                                        bj * BJ: bj * BJ + WIN_J]
                        nc.tensor.matmul(ptv[:, j, :NA], lhsA, rhsA,
                                         start=True, stop=False)
                        nc.tensor.matmul(ptv[:, j, :NA], lhsB, rhsB,
                                         start=False, stop=True)
                    t = bi * 2 + h
                    eng = (nc.vector.tensor_copy if COPY_ENG[t] == 0
                           else nc.scalar.copy)
                    dst = stage[:, h * TPB * BANDW:(h + 1) * TPB * BANDW]
                    dstv = dst.rearrange("p (b w) -> p b w", b=TPB)
                    eng(dstv, ptv[:, :, 0:BANDW])
                q = nc.scalar if bi % 2 == 0 else nc.sync
                q.dma_start(out=gout_ap[bi], in_=stage)
    nc.finalize()
    return nc


def _get_compiled():
    global _compiled
    if _compiled is None:
        _compiled = _build_bass()
    return _compiled


def _shard_inputs(src, tgt):
    """Per-core input maps: block-reorder src with zero strips, pad tgt."""
    import ml_dtypes

    bf16 = ml_dtypes.bfloat16
    in_maps = []
    for c in range(NCORES):
        b = c // 2
        r0 = HS * (c % 2)
        # [C, NBI, BI, NBJ, BJ] -> [C, (NBI NBJ), (BI BJ)] pixel blocks
        s = (src[b, :, r0:r0 + HS, :]
             .reshape(C, NBI, BI, NBJ, BJ)
             .transpose(0, 1, 3, 2, 4)
             .reshape(C, NBI * NBJ, BI * BJ))
        sz = np.zeros((C, NBI * NBJ, BLKC), dtype=np.float32)
        sz[:, :, 0:64] = s[:, :, 0:64]      # pixA (mi 0..3)
        sz[:, :, 128:192] = s[:, :, 64:128]  # pixB (mi 4..7)
        tp = np.zeros((C, TH, TW), dtype=np.float32)
        # window for output pixel (i, j) covers tgt rows i-2R..i and cols
        # j-2R..j; shard row q holds tgt row r0+q-2R; shard col x holds
        # tgt col x-2R.
        lo = r0 - 2 * R
        hi = r0 + HS
        clo = max(lo, 0)
        tp[:, clo - lo: clo - lo + (hi - clo), 2 * R: 2 * R + W] = \
            tgt[b, :, clo:hi, :]
        inp = np.concatenate([sz.reshape(C, SRCC), tp.reshape(C, TGTC)],
                             axis=1)
        in_maps.append({"inp": np.ascontiguousarray(inp.astype(bf16))})
    return in_maps


# host-side gather indices: out[k=(di,dj)] at pixel (mi,mj) of a block sits
# at band col n = (mi+di)*WIN_J + (mj+dj), shifted by BANDO for mi >= 4
# (their band holds window rows 4..15).
_mi = np.arange(BI)[:, None, None, None]
_mj = np.arange(BJ)[None, :, None, None]
_di = np.arange(D)[None, None, :, None]
_dj = np.arange(D)[None, None, None, :]
_NIDX = ((_mi + _di) * WIN_J + (_mj + _dj)
         - BANDO * (_mi >= 4)).reshape(BI, BJ, D * D)  # [8,16,81]


def _unshard_output(results):
    out = np.empty((B, D * D, H, W), dtype=np.float32)
    for c in range(NCORES):
        b = c // 2
        r0 = HS * (c % 2)
        g = (results[c]["gout"]
             .astype(np.float32)
             .reshape(NBI, 128, NBJ, BANDW)   # [bi, pixel, bj, band]
             .transpose(0, 2, 1, 3)
             .reshape(NBI, NBJ, BI, BJ, BANDW))
        # gather: v[bi,bj,mi,mj,k] = g[bi,bj,mi,mj,_NIDX[mi,mj,k]]
        v = np.take_along_axis(g, _NIDX[None, None], axis=-1)
        v = v.transpose(4, 0, 2, 1, 3)  # [81, NBI, BI, NBJ, BJ]
        out[b, :, r0:r0 + HS, :] = v.reshape(D * D, HS, W)
    return out


def kernel(src, tgt):
    from concourse.bass_utils import run_bass_kernel_spmd

    src = np.asarray(src, dtype=np.float32)
    tgt = np.asarray(tgt, dtype=np.float32)
    nc = _get_compiled()
    in_maps = _shard_inputs(src, tgt)
    res = run_bass_kernel_spmd(nc, in_maps, core_ids=list(range(NCORES)))
    return _unshard_output(res.results)
